# revision 3
# baseline (speedup 1.0000x reference)
"""Trainium2 Bass kernel for nn_BehavioralCircuit — pipelined 3-stage solver.

Reference: T=100000 sequential steps of a reward-modulated Hebbian rule over
512 independent 2-D units:
    r[t] = rewards[t] - movavg10(rewards)[t];  u = LR*r
    h    = sigmoid(W @ x_t);  m[t] = h.mean();  W += u[t] * outer(h, x_t)

Scheme (validated in sim + against an exact numpy mirror of the device
arithmetic; full-run rel err ~7.6e-4):
  Blocks of TAU=126 steps.  Within block b (64 units per core):
    seed : h0 = sigmoid(X_b.W(b-1))      [2-stale W + mtil2@h1(b-2) base]
    mid  : h1 = sigmoid(X_b.W(b-1) + mtil@h1(b-1) + K@h0)
    final: h2 = sigmoid(X_b.W(b) + K@h1) [exact base via mtil@(h2-h1)(b-1)]
    W(b) = W(b-1) + c(b-1)^T h2(b-1),  c = u*x
  K[t,s] = u_s (x_t.x_s) [s<t]; mtil/mtil2 couple adjacent blocks.  All
  per-block lhsT tiles are precomputed on the host and DMA-streamed.

Device per block: 7 matmuls (mtil2@h1(b-2), X@W stale (K=2), (c|mtil)@h1(b-1),
K@h0, K@(-h0), K@h1, (c|mtil)@d21(b-1)), 3 sigmoids on ACT, DVE: d21/neg/
W-updates.  h2 tiles are DMA'd out; the host does the final unit-mean.
The only cross-block serial chain is sigma2 -> d21 -> comb2d -> sigma2; the
W bookkeeping and the seed/mid stages are pipelined 1-2 blocks ahead.

All tiles are full 128-partition, base 0 (HW restriction on partition
offsets): rows 0:2 carry dW = c^T@h (or junk on h tiles, killed by zero
rows/cols in the streamed lhsT tiles), rows 2:128 are the 126 steps.
"""

import sys

import numpy as np

sys.path.insert(0, "/opt/trn_rl_repo")

import concourse.bass as bass
import concourse.bacc as bacc
import concourse.tile as tile
from concourse import mybir
from concourse.bass_utils import run_bass_kernel_spmd

TAU = 126
MP = 128            # tile partition size = 2 (dW rows) + TAU
T_FULL = 100000
NB_FULL = (T_FULL + TAU - 1) // TAU   # 794
NH = 512
NCORES = 8
UH = NH // NCORES   # 64
CH = 32             # stream chunk, blocks per DMA
LR = 0.1
WINDOW = 10

F32 = mybir.dt.float32
F16 = mybir.dt.float16
AF = mybir.ActivationFunctionType
OP = mybir.AluOpType


# ---------------------------------------------------------------------------
# Host-side stream preparation (shared across cores)
# ---------------------------------------------------------------------------

def _movavg_u(rewards, t_pad):
    cs = np.cumsum(rewards, dtype=np.float64)
    sh = np.concatenate([np.zeros(WINDOW), cs[:-WINDOW]])
    wsum = cs - sh
    count = np.minimum(np.arange(len(rewards)) + 1.0, float(WINDOW))
    u = (LR * (rewards - wsum / count)).astype(np.float32)
    up = np.zeros((t_pad,), np.float32)
    up[: len(rewards)] = u
    return up


def prep_streams(X, rewards, nb):
    t_pad = nb * TAU
    Xp = np.zeros((t_pad, 2), np.float32)
    Xp[: X.shape[0]] = X
    up = _movavg_u(rewards, t_pad)
    Xb = Xp.reshape(nb, TAU, 2)              # [b, t, 2]
    ub = up.reshape(nb, TAU)                 # [b, t]

    # kT[b][2+s, 2+t] = u_b[s] * (x_bs . x_bt) * [s < t]; rows/cols 0:2 = 0
    G = np.einsum("btc,bsc->bst", Xb, Xb)            # [b, s, t]
    kTc = G * ub[:, :, None]                         # u_s * (xs.xt)
    smask = np.triu(np.ones((TAU, TAU), np.float32), 1)  # [s,t]: s<t
    kT = np.zeros((nb, MP, MP), np.float32)
    kT[:, 2:, 2:] = kTc * smask[None]
    kT = kT.astype(np.float16)

    # CM[b] (b>=1): lhsT, contraction over rows 2+t' (h1(b-1) rows):
    #   cols 0:2  = c(b-1)[t', :] = u_{b-1}[t'] * x_{b-1, t'}
    #   cols 2+t  = mtilT[t', t]  = u_{b-1}[t'] * (x_{b-1,t'} . x_{b,t})
    CM = np.zeros((nb, MP, MP), np.float32)
    CM[1:, 2:, 0:2] = ub[:-1, :, None] * Xb[:-1]
    cross1 = np.einsum("btc,bsc->bst", Xb[1:], Xb[:-1])   # [j, t', t]
    CM[1:, 2:, 2:] = cross1 * ub[:-1, :, None]
    CM = CM.astype(np.float16)

    # SM[b]: mtil2 lhsT; rhs = h1(b-2) tile (rows 0:2 of rhs are junk,
    # killed by zero rows here):  SM[b][2+t'', 2+t] = mtil2T
    SM = np.zeros((nb, MP, MP), np.float32)
    cross2 = np.einsum("btc,bsc->bst", Xb[2:], Xb[:-2])   # [j, t'', t]
    SM[2:, 2:, 2:] = cross2 * ub[:-2, :, None]
    SM = SM.astype(np.float16)

    # SW[b]: [2, 128] stale lhsT (X_b^T at cols 2:), rhs = w16(b-2) [2, UH]
    SW = np.zeros((nb, 2, MP), np.float32)
    SW[:, :, 2:] = Xb.transpose(0, 2, 1)
    SW = SW.astype(np.float16)

    # flatten to [part, nb*MP] streams
    kT_s = np.ascontiguousarray(kT.transpose(1, 0, 2).reshape(MP, nb * MP))
    CM_s = np.ascontiguousarray(CM.transpose(1, 0, 2).reshape(MP, nb * MP))
    SM_s = np.ascontiguousarray(SM.transpose(1, 0, 2).reshape(MP, nb * MP))
    SW_s = np.ascontiguousarray(SW.transpose(1, 0, 2).reshape(2, nb * MP))
    return kT_s, CM_s, SM_s, SW_s


# ---------------------------------------------------------------------------
# Device program
# ---------------------------------------------------------------------------

def build_nc(nb: int):
    nc = bacc.Bacc("TRN2", target_bir_lowering=False, debug=False)
    nch = (nb + CH - 1) // CH
    cols = nch * CH * MP
    kT_d = nc.declare_dram_parameter("kT", [MP, cols], F16, isOutput=False)
    CM_d = nc.declare_dram_parameter("CM", [MP, cols], F16, isOutput=False)
    SM_d = nc.declare_dram_parameter("SM", [MP, cols], F16, isOutput=False)
    SW_d = nc.declare_dram_parameter("SW", [2, cols], F16, isOutput=False)
    w0_d = nc.declare_dram_parameter("w0T", [2, UH], F32, isOutput=False)
    h2o_d = nc.declare_dram_parameter("h2o", [MP, nb * UH], F16,
                                      isOutput=True)

    with tile.TileContext(nc) as tc:
        _emit(tc, nc, nb, nch, kT_d, CM_d, SM_d, SW_d, w0_d, h2o_d)
    nc.compile()
    return nc


def _emit(tc, nc, nb, nch, kT_d, CM_d, SM_d, SW_d, w0_d, h2o_d):
    from contextlib import ExitStack
    with ExitStack() as ctx:
        singles = ctx.enter_context(tc.tile_pool(name="singles", bufs=1))
        pool_kt = ctx.enter_context(tc.tile_pool(name="ktc", bufs=3))
        pool_cm = ctx.enter_context(tc.tile_pool(name="cmc", bufs=3))
        pool_sm = ctx.enter_context(tc.tile_pool(name="smc", bufs=3))
        pool_sw = ctx.enter_context(tc.tile_pool(name="swc", bufs=3))
        pool_r = ctx.enter_context(tc.tile_pool(name="rbuf", bufs=6))
        pool_h0 = ctx.enter_context(tc.tile_pool(name="h0buf", bufs=4))
        pool_h0n = ctx.enter_context(tc.tile_pool(name="h0nbuf", bufs=4))
        pool_h2 = ctx.enter_context(tc.tile_pool(name="h2buf", bufs=8))
        pool_d21 = ctx.enter_context(tc.tile_pool(name="d21buf", bufs=4))
        pool_wh = ctx.enter_context(tc.tile_pool(name="whbuf", bufs=4))
        pool_w16 = ctx.enter_context(tc.tile_pool(name="w16buf", bufs=4))
        psum_a = ctx.enter_context(tc.tile_pool(name="psa", bufs=6,
                                                space="PSUM"))

        w0_sb = singles.tile([2, UH], F32)
        nc.sync.dma_start(out=w0_sb, in_=w0_d[:, :])

        kt_ch, cm_ch, sm_ch, sw_ch = {}, {}, {}, {}

        def load_chunk(j):
            if j >= nch:
                return
            s = j * CH * MP
            e = (j + 1) * CH * MP
            kt = pool_kt.tile([MP, CH * MP], F16, tag="kt")
            nc.sync.dma_start(out=kt, in_=kT_d[:, s:e])
            cm = pool_cm.tile([MP, CH * MP], F16, tag="cm")
            nc.sync.dma_start(out=cm, in_=CM_d[:, s:e])
            sm = pool_sm.tile([MP, CH * MP], F16, tag="sm")
            nc.sync.dma_start(out=sm, in_=SM_d[:, s:e])
            sw = pool_sw.tile([2, CH * MP], F16, tag="sw")
            nc.sync.dma_start(out=sw, in_=SW_d[:, s:e])
            kt_ch[j], cm_ch[j], sm_ch[j], sw_ch[j] = kt, cm, sm, sw

        def st_ap(store, b):
            o = (b % CH) * MP
            return store[b // CH][:, o:o + MP]

        load_chunk(0)
        load_chunk(1)

        # initial R (zero) / w16 (fp16 W0) tiles and the f32 W master
        r_init0 = pool_r.tile([MP, UH], F16, tag="r")
        nc.vector.memset(r_init0, 0.0)
        r_init1 = pool_r.tile([MP, UH], F16, tag="r")
        nc.vector.memset(r_init1, 0.0)
        w16_init0 = pool_w16.tile([2, UH], F16, tag="w16")
        nc.vector.tensor_copy(w16_init0, w0_sb)
        w16_init1 = pool_w16.tile([2, UH], F16, tag="w16")
        nc.vector.tensor_copy(w16_init1, w0_sb)
        wh_init = pool_wh.tile([2, UH], F32, tag="wh")
        nc.vector.tensor_copy(wh_init, w0_sb)

        A, R, H0, H0N, H2, WH, D21, W16 = {}, {}, {}, {}, {}, {}, {}, {}
        R[-2], R[-1] = r_init0, r_init1
        W16[-2], W16[-1] = w16_init0, w16_init1
        WH[-1] = wh_init

        def mm_sm(b):
            # big early part: mtil2 @ h1(b-2), group start
            a = psum_a.tile([MP, UH], F32, tag="a")
            A[b] = a
            nc.tensor.matmul(a, lhsT=st_ap(sm_ch, b), rhs=R[b - 2],
                             start=True, stop=False, skip_group_check=True)

        def mm_sw(b):
            # tiny stale part: X_b @ W(b-2) via K=2 matmul
            nc.tensor.matmul(A[b], lhsT=st_ap(sw_ch, b), rhs=W16[b - 2],
                             start=False, stop=False, skip_group_check=True)

        def sig0(b):
            h0 = pool_h0.tile([MP, UH], F16, tag="h0")
            H0[b] = h0
            nc.scalar.activation(h0, A[b], AF.Sigmoid)

        def neg_h0(b):
            h0n = pool_h0n.tile([MP, UH], F16, tag="h0n")
            H0N[b] = h0n
            nc.vector.tensor_scalar(h0n, H0[b], -1.0, None, OP.mult)

        def mm_kh0(b):
            nc.tensor.matmul(A[b], lhsT=st_ap(kt_ch, b), rhs=H0[b],
                             start=False, stop=False, skip_group_check=True)

        def mm_comb1(b):
            nc.tensor.matmul(A[b], lhsT=st_ap(cm_ch, b), rhs=R[b - 1],
                             start=False, stop=False, skip_group_check=True)

        def sig1(b):
            r = pool_r.tile([MP, UH], F16, tag="r")
            R[b] = r
            nc.scalar.activation(r, A[b], AF.Sigmoid)

        def mm_knh0(b):
            nc.tensor.matmul(A[b], lhsT=st_ap(kt_ch, b), rhs=H0N[b],
                             start=False, stop=False, skip_group_check=True)

        def mm_kh1(b, stop=False):
            nc.tensor.matmul(A[b], lhsT=st_ap(kt_ch, b), rhs=R[b],
                             start=False, stop=stop, skip_group_check=True)

        def mm_comb2d(b):
            nc.tensor.matmul(A[b], lhsT=st_ap(cm_ch, b), rhs=D21[b - 1],
                             start=False, stop=True, skip_group_check=True)

        def sig2(b):
            h2 = pool_h2.tile([MP, UH], F16, tag="h2")
            H2[b] = h2
            nc.scalar.activation(h2, A[b], AF.Sigmoid)

        def dma_h2(b):
            nc.sync.dma_start(out=h2o_d[:, b * UH:(b + 1) * UH], in_=H2[b])

        def dve_d21(b):
            d = pool_d21.tile([MP, UH], F16, tag="d21")
            D21[b] = d
            nc.vector.tensor_tensor(d, H2[b], R[b], OP.subtract)

        def dve_whadd(b):
            wh = pool_wh.tile([2, UH], F32, tag="wh")
            nc.vector.tensor_tensor(wh, WH[b - 1], A[b][0:2, :], OP.add)
            WH[b] = wh

        def dve_whcopy(b):
            w = pool_w16.tile([2, UH], F16, tag="w16")
            W16[b] = w
            nc.vector.tensor_tensor(w, WH[b - 1], A[b][0:2, :], OP.add)

        # ---- prologue: blocks 0 and 1 seeded ----
        mm_sm(0)
        mm_sw(0)
        mm_sm(1)
        mm_sw(1)
        sig0(0)
        neg_h0(0)
        mm_kh0(0)
        sig1(0)

        EST = 0.001
        for b in range(nb):
            tc.tile_set_cur_wait(b * EST)
            if b % CH == 0:
                load_chunk(b // CH + 2)
            if b >= 1:
                dve_d21(b - 1)
            mm_knh0(b)
            mm_kh1(b, stop=(b == 0))
            if b >= 1:
                mm_comb2d(b)
            if b + 1 < nb:
                sig0(b + 1)
                mm_comb1(b + 1)
                mm_kh0(b + 1)
            sig2(b)
            dma_h2(b)
            if b + 1 < nb:
                dve_whcopy(b)
            dve_whadd(b)
            if b + 1 < nb:
                neg_h0(b + 1)
            if b + 2 < nb:
                mm_sm(b + 2)
                mm_sw(b + 2)
            if b + 1 < nb:
                sig1(b + 1)
            for d, store in ((4, A), (6, R), (3, H0), (3, H0N), (7, H2),
                             (3, D21), (3, WH), (3, W16)):
                store.pop(b - d, None)


# ---------------------------------------------------------------------------
# Host wrapper
# ---------------------------------------------------------------------------

def run_cores(X, rewards, W0, nb, t_real, trace=False):
    kT_s, CM_s, SM_s, SW_s = prep_streams(X, rewards, nb)
    nch = (nb + CH - 1) // CH
    cols = nch * CH * MP

    def pad(a):
        if a.shape[1] < cols:
            b = np.zeros((a.shape[0], cols), a.dtype)
            b[:, :a.shape[1]] = a
            return b
        return a

    kT_s, CM_s, SM_s, SW_s = pad(kT_s), pad(CM_s), pad(SM_s), pad(SW_s)

    nc = build_nc(nb)
    in_maps = []
    for c in range(NCORES):
        w0c = np.ascontiguousarray(W0[c * UH:(c + 1) * UH].T)  # [2, UH] f32
        in_maps.append({"kT": kT_s, "CM": CM_s, "SM": SM_s, "SW": SW_s,
                        "w0T": w0c.astype(np.float32)})
    res = run_bass_kernel_spmd(nc, in_maps, list(range(NCORES)), trace=trace)
    total = np.zeros((TAU, nb), np.float64)
    for c in range(NCORES):
        h2o = res.results[c]["h2o"]                  # [MP, nb*UH] f16
        total += h2o[2:MP].reshape(TAU, nb, UH).astype(np.float64).sum(axis=2)
    m = (total / float(NH)).T.reshape(-1)[:t_real].astype(np.float32)
    return m, res


def kernel(X, rewards, W_plastic_init):
    m, _ = run_cores(np.asarray(X, np.float32),
                     np.asarray(rewards, np.float32),
                     np.asarray(W_plastic_init, np.float32),
                     NB_FULL, T_FULL)
    return m


# revision 4
# speedup vs baseline: 1.1137x; 1.1137x over previous
"""Trainium2 Bass kernel for nn_BehavioralCircuit — pipelined 3-stage solver.

Reference: T=100000 sequential steps of a reward-modulated Hebbian rule over
512 independent 2-D units:
    r[t] = rewards[t] - movavg10(rewards)[t];  u = LR*r
    h    = sigmoid(W @ x_t);  m[t] = h.mean();  W += u[t] * outer(h, x_t)

Scheme (validated in sim + against an exact numpy mirror of the device
arithmetic; full-run rel err ~7.6e-4):
  Blocks of TAU=126 steps.  Within block b (64 units per core):
    seed : h0 = sigmoid(X_b.W(b-1))      [2-stale W + mtil2@h1(b-2) base]
    mid  : h1 = sigmoid(X_b.W(b-1) + mtil@h1(b-1) + K@h0)
    final: h2 = sigmoid(X_b.W(b) + K@h1) [exact base via mtil@(h2-h1)(b-1)]
    W(b) = W(b-1) + c(b-1)^T h2(b-1),  c = u*x
  K[t,s] = u_s (x_t.x_s) [s<t]; mtil/mtil2 couple adjacent blocks.  All
  per-block lhsT tiles are precomputed on the host and DMA-streamed.

Device per block: 7 matmuls (mtil2@h1(b-2), X@W stale (K=2), (c|mtil)@h1(b-1),
K@h0, K@(-h0), K@h1, (c|mtil)@d21(b-1)), 3 sigmoids on ACT, DVE: d21/neg/
W-updates.  h2 tiles are DMA'd out; the host does the final unit-mean.
The only cross-block serial chain is sigma2 -> d21 -> comb2d -> sigma2; the
W bookkeeping and the seed/mid stages are pipelined 1-2 blocks ahead.

All tiles are full 128-partition, base 0 (HW restriction on partition
offsets): rows 0:2 carry dW = c^T@h (or junk on h tiles, killed by zero
rows/cols in the streamed lhsT tiles), rows 2:128 are the 126 steps.
"""

import sys

import numpy as np

sys.path.insert(0, "/opt/trn_rl_repo")

import concourse.bass as bass
import concourse.bacc as bacc
import concourse.tile as tile
from concourse import mybir
from concourse.bass_utils import run_bass_kernel_spmd

TAU = 126
MP = 128            # tile partition size = 2 (dW rows) + TAU
T_FULL = 100000
NB_FULL = (T_FULL + TAU - 1) // TAU   # 794
NH = 512
NCORES = 8
UH = NH // NCORES   # 64
CH = 32             # stream chunk, blocks per DMA
LR = 0.1
WINDOW = 10

F32 = mybir.dt.float32
F16 = mybir.dt.float16
AF = mybir.ActivationFunctionType
OP = mybir.AluOpType


# ---------------------------------------------------------------------------
# Host-side stream preparation (shared across cores)
# ---------------------------------------------------------------------------

def _movavg_u(rewards, t_pad):
    cs = np.cumsum(rewards, dtype=np.float64)
    sh = np.concatenate([np.zeros(WINDOW), cs[:-WINDOW]])
    wsum = cs - sh
    count = np.minimum(np.arange(len(rewards)) + 1.0, float(WINDOW))
    u = (LR * (rewards - wsum / count)).astype(np.float32)
    up = np.zeros((t_pad,), np.float32)
    up[: len(rewards)] = u
    return up


def prep_streams(X, rewards, nb):
    t_pad = nb * TAU
    Xp = np.zeros((t_pad, 2), np.float32)
    Xp[: X.shape[0]] = X
    up = _movavg_u(rewards, t_pad)
    Xb = Xp.reshape(nb, TAU, 2)              # [b, t, 2]
    ub = up.reshape(nb, TAU)                 # [b, t]

    # kT[b][2+s, 2+t] = u_b[s] * (x_bs . x_bt) * [s < t]; rows/cols 0:2 = 0
    G = np.einsum("btc,bsc->bst", Xb, Xb)            # [b, s, t]
    kTc = G * ub[:, :, None]                         # u_s * (xs.xt)
    smask = np.triu(np.ones((TAU, TAU), np.float32), 1)  # [s,t]: s<t
    kT = np.zeros((nb, MP, MP), np.float32)
    kT[:, 2:, 2:] = kTc * smask[None]
    kT = kT.astype(np.float16)

    # CM[b] (b>=1): lhsT, contraction over rows 2+t' (h1(b-1) rows):
    #   cols 0:2  = c(b-1)[t', :] = u_{b-1}[t'] * x_{b-1, t'}
    #   cols 2+t  = mtilT[t', t]  = u_{b-1}[t'] * (x_{b-1,t'} . x_{b,t})
    CM = np.zeros((nb, MP, MP), np.float32)
    CM[1:, 2:, 0:2] = ub[:-1, :, None] * Xb[:-1]
    cross1 = np.einsum("btc,bsc->bst", Xb[1:], Xb[:-1])   # [j, t', t]
    CM[1:, 2:, 2:] = cross1 * ub[:-1, :, None]
    CM = CM.astype(np.float16)

    # SM[b]: mtil2 lhsT; rhs = h1(b-2) tile (rows 0:2 of rhs are junk,
    # killed by zero rows here):  SM[b][2+t'', 2+t] = mtil2T
    SM = np.zeros((nb, MP, MP), np.float32)
    cross2 = np.einsum("btc,bsc->bst", Xb[2:], Xb[:-2])   # [j, t'', t]
    SM[2:, 2:, 2:] = cross2 * ub[:-2, :, None]
    SM = SM.astype(np.float16)

    # SW[b]: [2, 128] stale lhsT (X_b^T at cols 2:), rhs = w16(b-2) [2, UH]
    SW = np.zeros((nb, 2, MP), np.float32)
    SW[:, :, 2:] = Xb.transpose(0, 2, 1)
    SW = SW.astype(np.float16)

    # flatten to [part, nb*MP] streams
    kT_s = np.ascontiguousarray(kT.transpose(1, 0, 2).reshape(MP, nb * MP))
    CM_s = np.ascontiguousarray(CM.transpose(1, 0, 2).reshape(MP, nb * MP))
    SM_s = np.ascontiguousarray(SM.transpose(1, 0, 2).reshape(MP, nb * MP))
    SW_s = np.ascontiguousarray(SW.transpose(1, 0, 2).reshape(2, nb * MP))
    return kT_s, CM_s, SM_s, SW_s


# ---------------------------------------------------------------------------
# Device program
# ---------------------------------------------------------------------------

def build_nc(nb: int):
    nc = bacc.Bacc("TRN2", target_bir_lowering=False, debug=False)
    nch = (nb + CH - 1) // CH
    cols = nch * CH * MP
    kT_d = nc.declare_dram_parameter("kT", [MP, cols], F16, isOutput=False)
    CM_d = nc.declare_dram_parameter("CM", [MP, cols], F16, isOutput=False)
    SM_d = nc.declare_dram_parameter("SM", [MP, cols], F16, isOutput=False)
    SW_d = nc.declare_dram_parameter("SW", [2, cols], F16, isOutput=False)
    w0_d = nc.declare_dram_parameter("w0T", [2, UH], F32, isOutput=False)
    h2o_d = nc.declare_dram_parameter("h2o", [MP, nb * UH], F16,
                                      isOutput=True)

    with tile.TileContext(nc) as tc:
        _emit(tc, nc, nb, nch, kT_d, CM_d, SM_d, SW_d, w0_d, h2o_d)
    nc.compile()
    return nc


def _emit(tc, nc, nb, nch, kT_d, CM_d, SM_d, SW_d, w0_d, h2o_d):
    from contextlib import ExitStack
    with ExitStack() as ctx:
        singles = ctx.enter_context(tc.tile_pool(name="singles", bufs=1))
        pool_kt = ctx.enter_context(tc.tile_pool(name="ktc", bufs=3))
        pool_cm = ctx.enter_context(tc.tile_pool(name="cmc", bufs=3))
        pool_sm = ctx.enter_context(tc.tile_pool(name="smc", bufs=3))
        pool_sw = ctx.enter_context(tc.tile_pool(name="swc", bufs=3))
        pool_r = ctx.enter_context(tc.tile_pool(name="rbuf", bufs=6))
        pool_h0 = ctx.enter_context(tc.tile_pool(name="h0buf", bufs=4))
        pool_h0n = ctx.enter_context(tc.tile_pool(name="h0nbuf", bufs=4))
        pool_h2 = ctx.enter_context(tc.tile_pool(name="h2buf", bufs=8))
        pool_d21 = ctx.enter_context(tc.tile_pool(name="d21buf", bufs=4))
        pool_wh = ctx.enter_context(tc.tile_pool(name="whbuf", bufs=4))
        pool_w16 = ctx.enter_context(tc.tile_pool(name="w16buf", bufs=4))
        psum_a = ctx.enter_context(tc.tile_pool(name="psa", bufs=6,
                                                space="PSUM"))

        w0_sb = singles.tile([2, UH], F32)
        nc.sync.dma_start(out=w0_sb, in_=w0_d[:, :])

        kt_ch, cm_ch, sm_ch, sw_ch = {}, {}, {}, {}

        def load_chunk(j):
            if j >= nch:
                return
            s = j * CH * MP
            e = (j + 1) * CH * MP
            kt = pool_kt.tile([MP, CH * MP], F16, tag="kt")
            nc.sync.dma_start(out=kt, in_=kT_d[:, s:e])
            cm = pool_cm.tile([MP, CH * MP], F16, tag="cm")
            nc.sync.dma_start(out=cm, in_=CM_d[:, s:e])
            sm = pool_sm.tile([MP, CH * MP], F16, tag="sm")
            nc.sync.dma_start(out=sm, in_=SM_d[:, s:e])
            sw = pool_sw.tile([2, CH * MP], F16, tag="sw")
            nc.sync.dma_start(out=sw, in_=SW_d[:, s:e])
            kt_ch[j], cm_ch[j], sm_ch[j], sw_ch[j] = kt, cm, sm, sw

        def st_ap(store, b):
            o = (b % CH) * MP
            return store[b // CH][:, o:o + MP]

        load_chunk(0)
        load_chunk(1)

        # initial R (zero) / w16 (fp16 W0) tiles and the f32 W master
        r_init0 = pool_r.tile([MP, UH], F16, tag="r")
        nc.vector.memset(r_init0, 0.0)
        r_init1 = pool_r.tile([MP, UH], F16, tag="r")
        nc.vector.memset(r_init1, 0.0)
        w16_init0 = pool_w16.tile([2, UH], F16, tag="w16")
        nc.vector.tensor_copy(w16_init0, w0_sb)
        w16_init1 = pool_w16.tile([2, UH], F16, tag="w16")
        nc.vector.tensor_copy(w16_init1, w0_sb)
        wh_init = pool_wh.tile([2, UH], F32, tag="wh")
        nc.vector.tensor_copy(wh_init, w0_sb)

        A, R, H0, H0N, H2, WH, D21, W16 = {}, {}, {}, {}, {}, {}, {}, {}
        R[-2], R[-1] = r_init0, r_init1
        W16[-2], W16[-1] = w16_init0, w16_init1
        WH[-1] = wh_init

        def mm_sm(b):
            # big early part: mtil2 @ h1(b-2), group start
            a = psum_a.tile([MP, UH], F32, tag="a")
            A[b] = a
            nc.tensor.matmul(a, lhsT=st_ap(sm_ch, b), rhs=R[b - 2],
                             start=True, stop=False, skip_group_check=True)

        def mm_sw(b):
            # tiny stale part: X_b @ W(b-2) via K=2 matmul
            nc.tensor.matmul(A[b], lhsT=st_ap(sw_ch, b), rhs=W16[b - 2],
                             start=False, stop=False, skip_group_check=True)

        def sig0(b):
            h0 = pool_h0.tile([MP, UH], F16, tag="h0")
            H0[b] = h0
            nc.scalar.activation(h0, A[b], AF.Sigmoid)

        def neg_h0(b):
            h0n = pool_h0n.tile([MP, UH], F16, tag="h0n")
            H0N[b] = h0n
            nc.vector.tensor_scalar(h0n, H0[b], -1.0, None, OP.mult)

        def mm_kh0(b):
            nc.tensor.matmul(A[b], lhsT=st_ap(kt_ch, b), rhs=H0[b],
                             start=False, stop=False, skip_group_check=True)

        def mm_comb1(b):
            nc.tensor.matmul(A[b], lhsT=st_ap(cm_ch, b), rhs=R[b - 1],
                             start=False, stop=False, skip_group_check=True)

        def sig1(b):
            r = pool_r.tile([MP, UH], F16, tag="r")
            R[b] = r
            nc.scalar.activation(r, A[b], AF.Sigmoid)

        def mm_knh0(b):
            nc.tensor.matmul(A[b], lhsT=st_ap(kt_ch, b), rhs=H0N[b],
                             start=False, stop=False, skip_group_check=True)

        def mm_kh1(b, stop=False):
            nc.tensor.matmul(A[b], lhsT=st_ap(kt_ch, b), rhs=R[b],
                             start=False, stop=stop, skip_group_check=True)

        def mm_comb2d(b):
            nc.tensor.matmul(A[b], lhsT=st_ap(cm_ch, b), rhs=D21[b - 1],
                             start=False, stop=True, skip_group_check=True)

        def sig2(b):
            h2 = pool_h2.tile([MP, UH], F16, tag="h2")
            H2[b] = h2
            nc.scalar.activation(h2, A[b], AF.Sigmoid)

        def dma_h2(b):
            nc.sync.dma_start(out=h2o_d[:, b * UH:(b + 1) * UH], in_=H2[b])

        def dve_d21(b):
            d = pool_d21.tile([MP, UH], F16, tag="d21")
            D21[b] = d
            nc.vector.tensor_tensor(d, H2[b], R[b], OP.subtract)

        def dve_whadd(b):
            wh = pool_wh.tile([2, UH], F32, tag="wh")
            nc.vector.tensor_tensor(wh, WH[b - 1], A[b][0:2, :], OP.add)
            WH[b] = wh

        def dve_whcopy(b):
            w = pool_w16.tile([2, UH], F16, tag="w16")
            W16[b] = w
            nc.vector.tensor_tensor(w, WH[b - 1], A[b][0:2, :], OP.add)

        # ---- prologue: blocks 0 and 1 seeded ----
        mm_sm(0)
        mm_sw(0)
        mm_sm(1)
        mm_sw(1)
        sig0(0)
        neg_h0(0)
        mm_kh0(0)
        sig1(0)

        EST = 0.001
        for b in range(nb):
            tc.tile_set_cur_wait(b * EST)
            if b % CH == 0:
                load_chunk(b // CH + 2)
            if b >= 1:
                dve_d21(b - 1)
            mm_knh0(b)
            mm_kh1(b, stop=(b == 0))
            if b >= 1:
                mm_comb2d(b)
            if b + 1 < nb:
                sig0(b + 1)
                mm_comb1(b + 1)
                mm_kh0(b + 1)
            sig2(b)
            dma_h2(b)
            dve_whadd(b)
            if b + 1 < nb:
                neg_h0(b + 1)
                dve_whcopy(b)
            if b + 2 < nb:
                mm_sm(b + 2)
                mm_sw(b + 2)
            if b + 1 < nb:
                sig1(b + 1)
            for d, store in ((4, A), (6, R), (3, H0), (3, H0N), (7, H2),
                             (3, D21), (3, WH), (3, W16)):
                store.pop(b - d, None)


# ---------------------------------------------------------------------------
# Host wrapper
# ---------------------------------------------------------------------------

def run_cores(X, rewards, W0, nb, t_real, trace=False):
    kT_s, CM_s, SM_s, SW_s = prep_streams(X, rewards, nb)
    nch = (nb + CH - 1) // CH
    cols = nch * CH * MP

    def pad(a):
        if a.shape[1] < cols:
            b = np.zeros((a.shape[0], cols), a.dtype)
            b[:, :a.shape[1]] = a
            return b
        return a

    kT_s, CM_s, SM_s, SW_s = pad(kT_s), pad(CM_s), pad(SM_s), pad(SW_s)

    nc = build_nc(nb)
    in_maps = []
    for c in range(NCORES):
        w0c = np.ascontiguousarray(W0[c * UH:(c + 1) * UH].T)  # [2, UH] f32
        in_maps.append({"kT": kT_s, "CM": CM_s, "SM": SM_s, "SW": SW_s,
                        "w0T": w0c.astype(np.float32)})
    res = run_bass_kernel_spmd(nc, in_maps, list(range(NCORES)), trace=trace)
    total = np.zeros((TAU, nb), np.float64)
    for c in range(NCORES):
        h2o = res.results[c]["h2o"]                  # [MP, nb*UH] f16
        total += h2o[2:MP].reshape(TAU, nb, UH).astype(np.float64).sum(axis=2)
    m = (total / float(NH)).T.reshape(-1)[:t_real].astype(np.float32)
    return m, res


def kernel(X, rewards, W_plastic_init):
    m, _ = run_cores(np.asarray(X, np.float32),
                     np.asarray(rewards, np.float32),
                     np.asarray(W_plastic_init, np.float32),
                     NB_FULL, T_FULL)
    return m
